# revision 19
# baseline (speedup 1.0000x reference)
"""AdaPT int8-quantized Linear on 8 TRN2 NeuronCores.

out = round_int8(x * 127/amax(x)) @ round_int8(w * 127/amax(w)).T * dequant + bias

Exactness: int8 values (|v| <= 127) are exact in bf16; their products
(<= 16129) and the accumulated partial sums (~1e5 << 2^24) are exact in
fp32 PSUM, so a bf16 TensorE matmul reproduces the int8 x int8 -> int32
matmul bit-exactly at full bf16 throughput. round() is implemented as
(v*scale + 1.5*2^23) - 1.5*2^23 in f32 (round-half-even, matching jnp).
All rounding-sensitive math stays on VectorE (ScalarE's activation affine
pre-op is not exact); abs/max are rounding-free.

Layout strategy: the TensorEngine contracts along the partition axis, so
both operands need k-major layout. kernel() passes x.T / w.T slices
(numpy prep); the device does only contiguous/strided loads, elementwise
quantize, matmuls and stores -- no device transposes, no large
collectives. Each core re-reads all of w.T and quantizes it
panel-by-panel, pipelined under the matmuls.

amax pipeline: per-core abs-max over disjoint slices (w first, then x,
via VectorE abs_max chains), exchanged in TWO tiny AllReduce-max
collectives so w quantization starts while the x scan is still running.

x row-parallel: core c computes out rows [c*1024, (c+1)*1024).
"""

import numpy as np

import concourse.bass as bass
import concourse.bacc as bacc
import concourse.bass_isa as bass_isa
import concourse.mybir as mybir
import concourse.tile as tile
from concourse.bass_utils import run_bass_kernel_spmd

N, K, M = 8192, 4096, 4096
N_CORES = 8
NS = N // N_CORES   # 1024 x rows per core
WS = M // N_CORES   # 512 w rows per core (amax shard)
P = 128
KB = K // P         # 32 k-blocks
NB = NS // P        # 8 n-blocks per core
MP = 512            # m-panel width
NMP = M // MP       # 8 m-panels

MAGIC = 12582912.0  # 1.5 * 2**23
F32 = mybir.dt.float32
BF16 = mybir.dt.bfloat16

_cached_nc = None


def _body(nc, tc, xs, wa, wf, bias_in, out):
    RG = [list(range(N_CORES))]
    # xs: [K, NS] f32 (x.T slice)   -> tiles [128, 4, NS], k on partitions
    # wa: [K, WS] f32 (w.T slice)   -> tiles [128, 8, MP] (amax only)
    # wf: [K, M]  f32 (full w.T)    -> per-panel tiles [128, 8, MP]
    xs_t = xs.rearrange("(t a p) n -> t p a n", a=4, p=P)   # [8, 128, 4, 1024]
    wa_t = wa.rearrange("(h j p) m -> h p j m", j=8, p=P)   # [4, 128, 8, 512]

    with (
        tc.tile_pool(name="const", bufs=1) as const,
        tc.tile_pool(name="dram", bufs=1, space="DRAM") as dram,
        tc.tile_pool(name="ld", bufs=4) as ld,
        tc.tile_pool(name="xt", bufs=1) as xtp,
        tc.tile_pool(name="wt", bufs=6) as wtp,
        tc.tile_pool(name="ps", bufs=4, space="PSUM") as psp,
        tc.tile_pool(name="ob", bufs=3) as obp,
    ):
        ccw_in = dram.tile([1, 16], F32)
        ccw_out = dram.tile([1, 16], F32, addr_space="Shared")
        ccx_in = dram.tile([1, 16], F32)
        ccx_out = dram.tile([1, 16], F32, addr_space="Shared")

        bias_bc = const.tile([P, M], F32)
        scl = const.tile([P, 4], F32)   # 0:scale_x 1:scale_w 2:dequant 3:tmp

        # ---- local abs-max: x slice first (8 tiles), then w slice (4) ----
        # AR_x goes out first so x-quantize (which gates the first matmul)
        # overlaps AR_w's flight on the collective stream.
        partw = const.tile([P, 4], F32)
        partx = const.tile([P, 8], F32)
        for t in range(8):
            tl = ld.tile([P, 4, NS], F32, tag="ldf32", name=f"ldx{t}")
            nc.sync.dma_start(tl[:], xs_t[t])
            nc.vector.tensor_reduce(
                out=partx[:, t : t + 1], in_=tl[:], op=mybir.AluOpType.max,
                axis=mybir.AxisListType.XY, apply_absolute_value=True,
            )
        px = const.tile([P, 1], F32)
        nc.vector.tensor_reduce(out=px[:], in_=partx[:], op=mybir.AluOpType.max,
                                axis=mybir.AxisListType.X)
        rx = const.tile([P, 1], F32)
        nc.gpsimd.partition_all_reduce(rx[:], px[:], channels=P,
                                       reduce_op=bass_isa.ReduceOp.max)
        packx = const.tile([1, 16], F32)
        nc.vector.memset(packx[:], 0.0)
        nc.vector.tensor_copy(packx[:1, 0:1], rx[:1, :])
        nc.gpsimd.dma_start(ccx_in[:], packx[:])
        nc.gpsimd.collective_compute(
            "AllReduce", mybir.AluOpType.max,
            ins=[ccx_in.opt()], outs=[ccx_out.opt()], replica_groups=RG,
        )

        for h in range(4):
            tl = ld.tile([P, 8, MP], F32, tag="ldf32", name=f"ldw{h}")
            nc.scalar.dma_start(tl[:], wa_t[h])
            nc.vector.tensor_reduce(
                out=partw[:, h : h + 1], in_=tl[:], op=mybir.AluOpType.max,
                axis=mybir.AxisListType.XY, apply_absolute_value=True,
            )
        pw = const.tile([P, 1], F32)
        nc.vector.tensor_reduce(out=pw[:], in_=partw[:], op=mybir.AluOpType.max,
                                axis=mybir.AxisListType.X)
        rw = const.tile([P, 1], F32)
        nc.gpsimd.partition_all_reduce(rw[:], pw[:], channels=P,
                                       reduce_op=bass_isa.ReduceOp.max)
        packw = const.tile([1, 16], F32)
        nc.vector.memset(packw[:], 0.0)
        nc.vector.tensor_copy(packw[:1, 0:1], rw[:1, :])
        nc.gpsimd.dma_start(ccw_in[:], packw[:])
        nc.gpsimd.collective_compute(
            "AllReduce", mybir.AluOpType.max,
            ins=[ccw_in.opt()], outs=[ccw_out.opt()], replica_groups=RG,
        )

        gotx = const.tile([1, 16], F32)
        nc.gpsimd.dma_start(gotx[:], ccx_out[:])
        gbx = const.tile([P, 16], F32)
        nc.gpsimd.partition_broadcast(gbx[:], gotx[:])
        invx = const.tile([P, 1], F32)
        nc.vector.reciprocal(invx[:], gbx[:, 0:1])
        nc.vector.tensor_scalar(out=scl[:, 0:1], in0=invx[:], scalar1=127.0,
                                scalar2=None, op0=mybir.AluOpType.mult)

        gotw = const.tile([1, 16], F32)
        nc.gpsimd.dma_start(gotw[:], ccw_out[:])
        gbw = const.tile([P, 16], F32)
        nc.gpsimd.partition_broadcast(gbw[:], gotw[:])
        invw = const.tile([P, 1], F32)
        nc.vector.reciprocal(invw[:], gbw[:, 0:1])
        nc.vector.tensor_scalar(out=scl[:, 1:2], in0=invw[:], scalar1=127.0,
                                scalar2=None, op0=mybir.AluOpType.mult)
        nc.vector.tensor_tensor(out=scl[:, 3:4], in0=gbx[:, 0:1], in1=gbw[:, 0:1],
                                op=mybir.AluOpType.mult)
        nc.vector.tensor_scalar(out=scl[:, 2:3], in0=scl[:, 3:4],
                                scalar1=float(np.float32(1.0) / np.float32(16129.0)),
                                scalar2=None, op0=mybir.AluOpType.mult)

        bias_b_ap = bass.AP(
            tensor=bias_in.tensor,
            offset=bias_in.offset,
            ap=[[0, P]] + list(bias_in.ap),
        )
        nc.gpsimd.dma_start(out=bias_bc[:], in_=bias_b_ap)

        xT = xtp.tile([P, KB, NS], BF16)  # resident quantized x.T (8.4 MB)

        def quant_w_chunk(p, h):
            tl = ld.tile([P, 8, MP], F32, tag="ldf32", name=f"ldwp{p}_{h}")
            src = bass.AP(
                tensor=wf.tensor,
                offset=wf.offset + h * (K // 4) * M + p * MP,
                ap=[[M, P], [P * M, 8], [1, MP]],
            )
            nc.scalar.dma_start(tl[:], src)
            nc.vector.tensor_scalar(out=tl[:], in0=tl[:], scalar1=scl[:, 1:2],
                                    scalar2=MAGIC, op0=mybir.AluOpType.mult,
                                    op1=mybir.AluOpType.add)
            w = wtp.tile([P, 8, MP], BF16, tag="wT", name=f"wT{p}_{h}")
            nc.vector.tensor_scalar(out=w[:], in0=tl[:], scalar1=MAGIC,
                                    scalar2=None, op0=mybir.AluOpType.subtract)
            return w

        def quant_x_tile(t):
            tl = ld.tile([P, 4, NS], F32, tag="ldf32", name=f"ldx2{t}")
            nc.sync.dma_start(tl[:], xs_t[t])
            nc.vector.tensor_scalar(out=tl[:], in0=tl[:], scalar1=scl[:, 0:1],
                                    scalar2=MAGIC, op0=mybir.AluOpType.mult,
                                    op1=mybir.AluOpType.add)
            nc.vector.tensor_scalar(out=xT[:, 4 * t : 4 * t + 4, :], in0=tl[:],
                                    scalar1=MAGIC, scalar2=None,
                                    op0=mybir.AluOpType.subtract)

        # ramp: x quantize runs while AR_w is still in flight
        for t in range(8):
            quant_x_tile(t)
        panel_w = {0: [quant_w_chunk(0, h) for h in range(4)]}

        # ---- main loop: matmuls + epilogue (w quant pipelined one ahead) ----
        for p in range(NMP):
            wth = panel_w.pop(p)
            if p + 1 < NMP:
                panel_w[p + 1] = [quant_w_chunk(p + 1, h) for h in range(4)]
            for nb in range(NB):
                ps = psp.tile([P, MP], F32, tag="ps", name=f"ps{p}_{nb}")
                for i in range(KB):
                    ks = (4 * nb + i) % KB
                    nc.tensor.matmul(
                        ps[:], xT[:, ks, nb * P : (nb + 1) * P],
                        wth[ks // 8][:, ks % 8, :],
                        start=(i == 0), stop=(i == KB - 1),
                    )
                ob = obp.tile([P, MP], F32, tag="ob", name=f"ob{p}_{nb}")
                nc.vector.scalar_tensor_tensor(
                    out=ob[:], in0=ps[:], scalar=scl[:, 2:3],
                    in1=bias_bc[:, p * MP : (p + 1) * MP],
                    op0=mybir.AluOpType.mult, op1=mybir.AluOpType.add,
                )
                nc.gpsimd.dma_start(
                    out[nb * P : (nb + 1) * P, p * MP : (p + 1) * MP], ob[:]
                )


def _build():
    global _cached_nc
    if _cached_nc is not None:
        return _cached_nc
    nc = bacc.Bacc("TRN2", target_bir_lowering=False, debug=False,
                   num_devices=N_CORES)
    xs = nc.dram_tensor("xs", [K, NS], F32, kind="ExternalInput")
    wa = nc.dram_tensor("wa", [K, WS], F32, kind="ExternalInput")
    wf = nc.dram_tensor("wf", [K, M], F32, kind="ExternalInput")
    bias = nc.dram_tensor("bias", [M], F32, kind="ExternalInput")
    out = nc.dram_tensor("out", [NS, M], F32, kind="ExternalOutput")
    with tile.TileContext(nc) as tc:
        _body(nc, tc, xs.ap(), wa.ap(), wf.ap(), bias.ap(), out.ap())
    nc.compile()
    _cached_nc = nc
    return nc


def kernel(x, weight, bias, _trace=False, _trace_kwargs=None):
    x = np.asarray(x, dtype=np.float32)
    weight = np.asarray(weight, dtype=np.float32)
    bias = np.ascontiguousarray(np.asarray(bias, dtype=np.float32))
    assert x.shape == (N, K) and weight.shape == (M, K) and bias.shape == (M,)

    nc = _build()
    xt = x.T                              # [K, N] view
    wt = np.ascontiguousarray(weight.T)   # [K, M]
    in_maps = [
        {
            "xs": np.ascontiguousarray(xt[:, c * NS : (c + 1) * NS]),
            "wa": np.ascontiguousarray(wt[:, c * WS : (c + 1) * WS]),
            "wf": wt,
            "bias": bias,
        }
        for c in range(N_CORES)
    ]
    res = run_bass_kernel_spmd(
        nc, in_maps, core_ids=list(range(N_CORES)),
        trace=_trace, **(_trace_kwargs or {}),
    )
    out = np.concatenate([res.results[c]["out"] for c in range(N_CORES)], axis=0)
    if _trace:
        return out, res
    return out
